# revision 10
# baseline (speedup 1.0000x reference)
"""Trainium2 Bass kernel for nn_EcholancerLoss (token CE + CTC forward-sum loss).

Sharding: data-parallel over batch B=8 (one batch item per NeuronCore) for the
token-CE logsumexp (the 143MB of logits dominate memory traffic). The CTC DP
over all 32 (batch, head) items is replicated on every core (it is latency-
bound, not throughput-bound, so replication costs no wall-clock and keeps the
program SPMD-uniform); host reads CTC outputs from core 0.

Per core:
  - Token CE: row-wise logsumexp over the audio vocab slice (1024 x 4096) via
    ScalarE exp+accumulate. Target-logit gather and the masked reduction are
    exact host-side numpy on the raw inputs.
  - CTC forward-sum: prob-space DP as affine recurrences evaluated with
    tensor_tensor_scan (25 time steps per instruction), parallelized as a
    wavefront over w = j + c with 128 partitions = (time-chunk c, item n).
    Chunk-boundary states cross partitions via a constant shift-by-4 matmul
    on TensorE (compute engines cannot address partition offsets != 0/32/64/96).
    A Viterbi (max-plus scan) pre-pass yields per-chunk rescale rates delta_c
    keeping fp32 in range; host applies exact log-corrections, so any delta
    gives identical results up to fp32 rounding.
"""

import numpy as np

B, H, TQ, TK = 8, 4, 800, 128
T_TOK, V_TEXT, V_TOTAL = 1024, 256, 4352
VA = V_TOTAL - V_TEXT
NEG = -1e9
BLANK = -8.0
CE_W, ATTN_W, ATTN_START = 1.5, 10.0, 5000
C, L = 32, 25            # time chunks x chunk length = 800
W = TK + C               # 160 wavefronts (covers even-state j=128)
NSLOT = W + 1            # slot 0 = virtual block -1
CE_TILES = T_TOK // 128  # 8
N_ITEMS = B * H

_CACHE = {}


def _build_nc():
    import concourse.bacc as bacc
    import concourse.mybir as mybir
    import concourse.tile as tile

    dt = mybir.dt.float32
    AF = mybir.ActivationFunctionType
    OP = mybir.AluOpType

    nc = bacc.Bacc("TRN2", target_bir_lowering=False, debug=False,
                   enable_asserts=False)
    ce_in = nc.dram_tensor("ce_in", [CE_TILES, 128, VA], dt,
                           kind="ExternalInput").ap()
    lp_in = nc.dram_tensor("lp_in", [128, W, L], dt, kind="ExternalInput").ap()
    sh_in = nc.dram_tensor("sh_in", [128, 128], dt, kind="ExternalInput").ap()
    kp_in = nc.dram_tensor("kp_in", [128, 1], dt, kind="ExternalInput").ap()
    lse_out = nc.dram_tensor("lse_out", [128, CE_TILES], dt,
                             kind="ExternalOutput").ap()
    m_out = nc.dram_tensor("m_out", [128, 1], dt, kind="ExternalOutput").ap()
    eo_out = nc.dram_tensor("eo_out", [128, NSLOT, 2, 26], dt,
                            kind="ExternalOutput").ap()

    with tile.TileContext(nc) as tc:
        with tc.tile_pool(name="main", bufs=1) as pool, \
             tc.tile_pool(name="ce", bufs=2) as cep, \
             tc.tile_pool(name="psum", bufs=4, space="PSUM") as psp:
            # ---------------- CTC setup ----------------
            LP = pool.tile([128, W, L], dt, tag="lp")
            nc.sync.dma_start(LP[:], lp_in)
            SH = pool.tile([128, 128], dt, tag="sh")
            nc.sync.dma_start(SH[:], sh_in)
            KP = pool.tile([128, 1], dt, tag="kp")
            nc.sync.dma_start(KP[:], kp_in)
            LPB = pool.tile([128, L], dt, tag="lpb")
            nc.vector.memset(LPB[:], BLANK)
            NEGC = pool.tile([128, 1], dt, tag="negc")
            nc.vector.memset(NEGC[:], NEG)
            E8 = pool.tile([128, 1], dt, tag="e8")
            nc.vector.memset(E8[:], -BLANK)
            NEG8 = pool.tile([128, L], dt, tag="neg8")
            nc.vector.memset(NEG8[:], BLANK)
            U = pool.tile([128, L], dt, tag="u")

            MEO = pool.tile([128, NSLOT, 2, 26], dt, tag="meo")
            EO = pool.tile([128, NSLOT, 2, 26], dt, tag="eo")
            # bulk fills on GpSimd (off the DVE/ACT critical paths)
            nc.gpsimd.memset(MEO[:], NEG)
            nc.gpsimd.memset(EO[:], 0.0)

            # ---------------- CE: row logsumexp ----------------
            sums = pool.tile([128, CE_TILES], dt, tag="sums")
            for i in range(CE_TILES):
                cet = cep.tile([128, VA], dt, tag="cet")
                scr = cep.tile([128, VA], dt, tag="scr")
                nc.sync.dma_start(cet[:], ce_in[i])
                nc.scalar.activation(scr[:], cet[:], AF.Exp,
                                     accum_out=sums[:, i:i + 1])
            lse = pool.tile([128, CE_TILES], dt, tag="lse")
            nc.scalar.activation(lse[:], sums[:], AF.Ln)
            nc.sync.dma_start(lse_out, lse[:])

            # ---------------- Viterbi (max-plus) pass ----------------
            for w in range(W):
                mm = psp.tile([128, 2], dt, tag="mm")
                nc.tensor.matmul(mm[:], SH[:], MEO[:, w, :, 25])
                nc.vector.tensor_copy(MEO[:, w + 1, :, 0], mm[:])
                nc.vector.memset(MEO[0:4, w + 1, :, 0], NEG)
                if w == 0:
                    nc.vector.memset(MEO[0:4, 1, 0, 0:1], 0.0)
                nc.vector.tensor_tensor_scan(
                    MEO[:, w + 1, 0, 1:26], MEO[:, w, 1, 0:25], LPB[:],
                    MEO[:, w + 1, 0, 0:1], op0=OP.max, op1=OP.add)
                nc.vector.tensor_tensor(U[:], MEO[:, w + 1, 0, 0:25],
                                        MEO[:, w, 1, 0:25], op=OP.max)
                nc.vector.tensor_tensor_scan(
                    MEO[:, w + 1, 1, 1:26], U[:], LP[:, w, :],
                    MEO[:, w + 1, 1, 0:1], op0=OP.max, op1=OP.add)

            # M_c from odd-state chunk-end maxima; delta_c = (M_c - M_{c-1})/L
            M = pool.tile([128, 1], dt, tag="m")
            nc.vector.tensor_reduce(M[:], MEO[:, :, 1, 25],
                                    axis=mybir.AxisListType.X, op=OP.max)
            nc.sync.dma_start(m_out, M[:])
            msh = psp.tile([128, 1], dt, tag="msh")
            nc.tensor.matmul(msh[:], SH[:], M[:])
            Dm = pool.tile([128, 1], dt, tag="dm")
            nc.vector.tensor_tensor(Dm[:], M[:], msh[:], op=OP.subtract)
            DS = pool.tile([128, 1], dt, tag="ds")
            nc.vector.tensor_scalar(DS[:], Dm[:], 1.0 / L, KP[:, 0:1],
                                    op0=OP.mult, op1=OP.add)
            ND = pool.tile([128, 1], dt, tag="nd")
            nc.scalar.mul(ND[:], DS[:], -1.0)
            IPB = pool.tile([128, 1], dt, tag="ipb")
            nc.scalar.activation(IPB[:], DS[:], AF.Exp, bias=E8[:, 0:1])
            P = pool.tile([128, W, L], dt, tag="p")
            nc.scalar.activation(P[:], LP[:], AF.Exp, bias=ND[:, 0:1])
            PB = pool.tile([128, L], dt, tag="pb")
            nc.scalar.activation(PB[:], NEG8[:], AF.Exp, bias=ND[:, 0:1])

            # ---------------- forward (prob-space) pass ----------------
            for w in range(W):
                mm = psp.tile([128, 2], dt, tag="mm")
                nc.tensor.matmul(mm[:], SH[:], EO[:, w, :, 25])
                nc.vector.tensor_copy(EO[:, w + 1, :, 0], mm[:])
                if w == 0:
                    nc.vector.memset(EO[0:4, 1, 0, 0:1], 1.0)
                nc.vector.tensor_tensor_scan(
                    EO[:, w + 1, 0, 1:26], EO[:, w, 1, 0:25], PB[:],
                    EO[:, w + 1, 0, 0:1], op0=OP.add, op1=OP.mult)
                nc.vector.tensor_scalar(U[:], EO[:, w + 1, 0, 1:26],
                                        IPB[:, 0:1], None, op0=OP.mult)
                nc.vector.tensor_tensor_scan(
                    EO[:, w + 1, 1, 1:26], U[:], P[:, w, :],
                    EO[:, w + 1, 1, 0:1], op0=OP.add, op1=OP.mult)

            nc.sync.dma_start(eo_out, EO[:])

    nc.compile()
    return nc


def _get_nc():
    if "nc" not in _CACHE:
        _CACHE["nc"] = _build_nc()
    return _CACHE["nc"]


def _shift_mat():
    s = np.zeros((128, 128), np.float32)
    # lhsT[k, m] = 1 iff k == m - 4  (out[m] = rhs[m-4])
    for m in range(4, 128):
        s[m - 4, m] = 1.0
    return s


def kappa_of_k(k):
    """Entropy-rate correction for the Viterbi-based rescale (nats/step)."""
    return 0.00113 * k - 0.0428 + 0.005


def make_in_maps(logits, attn, klens):
    """Host-side sharding: per-core CE slice + per-batch skewed CTC emissions."""
    sh = _shift_mat()
    in_maps = []
    for b in range(B):
        ce = np.ascontiguousarray(
            logits[b, :, V_TEXT:], dtype=np.float32).reshape(CE_TILES, 128, VA)
        am = np.where(np.arange(TK)[None, None, :] < klens[b],
                      attn[b], NEG).astype(np.float32)
        A2 = am.reshape(H, C, L, TK).transpose(1, 0, 3, 2)  # (c, n, j, tau)
        lp = np.full((128, W, L), NEG, np.float32)
        for c in range(C):
            lp[4 * c:4 * c + 4, c:c + TK, :] = A2[c]
        kp = np.full((128, 1), kappa_of_k(int(klens[b])), np.float32)
        in_maps.append({"ce_in": ce, "lp_in": lp, "sh_in": sh, "kp_in": kp})
    return in_maps


def finalize(results, logits, attn, tgts, alens, klens, qlens, step):
    """Host-side unshard + scalar reductions (exact)."""
    valid = np.arange(T_TOK)[None, :] < alens[:, None]
    lse_all = np.stack([r["lse_out"].T.reshape(-1) for r in results])  # (B,1024)
    x_tgt = np.take_along_axis(
        logits, tgts.astype(np.int64)[:, :, None], axis=2)[:, :, 0]
    denom = max(int(valid.sum()), 1)
    token_loss = float(np.sum(np.where(valid, lse_all - x_tgt, 0.0))) / denom

    if step > ATTN_START:
        am = np.where(np.arange(TK)[None, None, None, :] <
                      klens[:, None, None, None], attn, NEG)
        lpfull = np.concatenate(
            [np.full((B, H, TQ, 1), BLANK, np.float32), am], axis=3)
        mx = lpfull.max(axis=3)
        lse_t = mx + np.log(np.sum(np.exp(lpfull - mx[..., None]), axis=3))
        cum_lse = np.cumsum(lse_t.astype(np.float64), axis=2)

        losses = np.zeros((B, H), np.float64)
        for b in range(B):
            r = results[b]
            EO = r["eo_out"]
            m_chunk = r["m_out"][:, 0].astype(np.float64)
            k, q = int(klens[b]), int(qlens[b])
            t_s = q - 1
            c_s, tau_s = t_s // L, t_s % L
            kap = kappa_of_k(k)
            for h in range(H):
                p = 4 * c_s + h
                mcs = m_chunk[np.arange(C) * 4 + h]
                delta = np.empty(C, np.float64)
                delta[0] = mcs[0] / L + kap
                delta[1:] = (mcs[1:] - mcs[:-1]) / L + kap
                scale = L * delta[:c_s].sum() + (tau_s + 1) * delta[c_s]
                e1 = EO[p, (k - 1) + c_s + 1, 1, 1 + tau_s]
                e2 = EO[p, k + c_s + 1, 0, 1 + tau_s]
                with np.errstate(divide="ignore"):
                    la = np.logaddexp(np.log(e1), np.log(e2)) + scale \
                        - cum_lse[b, h, t_s]
                loss = -la / k
                if not (np.isfinite(loss) and loss < 1e8):
                    loss = 0.0
                losses[b, h] = loss
        attn_loss = float(losses.mean())
    else:
        attn_loss = 0.0

    total = token_loss * CE_W + attn_loss * ATTN_W
    return np.array([total, attn_loss, token_loss], np.float32)


def kernel(**inputs):
    from concourse.bass_utils import run_bass_kernel_spmd

    logits = np.asarray(inputs["logits"], np.float32)
    attn = np.asarray(inputs["attn_logprob"], np.float32)
    tgts = np.asarray(inputs["token_targets"])
    alens = np.asarray(inputs["audio_target_lens"]).astype(np.int64)
    slens = np.asarray(inputs["src_lens"]).astype(np.int64)
    olens = np.asarray(inputs["out_lens"]).astype(np.int64)
    step = int(np.asarray(inputs["current_step"]))
    klens = np.minimum(slens, TK)
    qlens = np.minimum(olens, TQ)

    nc = _get_nc()
    in_maps = make_in_maps(logits, attn, klens)
    res = run_bass_kernel_spmd(nc, in_maps, list(range(B)))
    return finalize(res.results, logits, attn, tgts, alens, klens, qlens, step)
